# revision 38
# baseline (speedup 1.0000x reference)
"""Grouped per-sample MLP (conv1d groups=B) + GroupSwish + softmax, on 8 NeuronCores.

Data-parallel over the group/batch axis B=256: 32 groups per core,
processed as 8 quads of 4 groups packed into the 128-partition dim.

Per group g: h = W1[g] @ x[g] + b1[g]; GroupSwish; o = W2[g] @ h + b2[g];
softmax over the flattened [C*L] logits.

The kernel is HBM-stream-bound (~290 GB/s/core under 8-core load, ~13.8MB
per core => ~48us stream floor); the design keeps the stream saturated
and the post-stream tail short:
  - x and W1 ship as fp8e4m3, swish output as fp16, out as bf16. End-to-
    end rel err ~9e-3 vs the 2e-2 gate.
  - The DMA queues' rates ADD (SWDGE alone ~245, sync HWDGE alone ~265,
    together ~292 GB/s), and a queue only delivers its share while it
    has work queued. So: the gpsimd queue carries 20 of each quad's 28
    512B (c,j) blocks as the in-order granule stream (FIFO ring order =
    consumption order); the sync queue carries W1 + consts + an 8-block
    share per quad issued ONE QUAD EARLY, so its structural backlog
    never delays the quad being computed. Stores ride the third queue
    (scalar HWDGE) so no x-issuing engine ever blocks on compute.
  - Q0 granule sizes are 2048B-aligned (mid-stream 8/8/4 units, last
    quad 4/4/4/4/2/2) so W1 matmuls consume x as it lands; after the
    final byte only the last small granule's matmuls remain.
  - Emission order per iteration interleaves the previous quads' late
    stages between matmul granule groups, so in-order engine queues
    never park a ready instruction behind a stalled matmul:
    PE:  [mmG0(q)] [W2(q-1)] [mmG1(q)] [tot(q-2)] [mmG2(q)] [mmG3(q)]
    DVE: [recip(q-2)] [mul(q-3)]
    ACT: [exp(q-1)] [silu(q)]
  - GroupSwish is ONE activation: silu(sp*(h+b1)) with per-partition
    scale/bias, and the 1/(1.1*sp) factor folded into W2 host-side.
  - W2 is a single block-diagonal [128, 40] fp16 matmul whose output
    lands compactly at partitions 10j+m, so softmax runs on [40, L]
    with no padding and the store is ONE plain [40, 512] DMA per quad.
  - All of W1 stays resident in SBUF; softplus(beta) and all folding are
    host-side. Softmax denominators via one [40,40] block-mask matmul.
"""

import os
import ml_dtypes
import numpy as np
from contextlib import ExitStack

import concourse.mybir as mybir
import concourse.tile as tile
from concourse import bacc
from concourse.bass_utils import run_bass_kernel_spmd

B, X, Z, C, L = 256, 784, 32, 10, 512
NCORE = 8
GPC = B // NCORE  # 32 groups per core
NQ = GPC // 4  # 8 quads per core
KC = 112  # K-chunk size (7 * 112 = 784)
NCH = 7
P = 128
NB = NCH * 4  # 28 512B (c,j) blocks per quad per partition
F32 = mybir.dt.float32
F16 = mybir.dt.float16
F8 = mybir.dt.float8e4
BF16 = mybir.dt.bfloat16

SU = 12   # of 28 x blocks/quad on the sync (HWDGE) queue (3 chunks)
NG = 4    # gpsimd granules per quad (one chunk each, 2048B rows)

DEFAULT_CFG = dict(
    x_bufs=5,
    s_bufs=4,
    h_bufs=3,
    o_bufs=2,
    prefetch=1,      # quads of lead for the sync share (absorbs backlog)
    x_engine="gpsimd",   # Q0: in-order granule stream (bulk of x)
    aux_engine="sync",   # Q1: W1 + consts + per-quad x share + stores
    out_engine="sync",
)

_CACHE: dict = {}


def _eng(nc, name):
    return getattr(nc, name)


def _build(cfg=DEFAULT_CFG):
    nc = bacc.Bacc("TRN2", target_bir_lowering=False, debug=False)

    # every DMA source below is a fully contiguous DRAM block — strided
    # row-slices cost ~10% of aggregate HBM rate (measured)
    xs = nc.dram_tensor("xs", [NQ, KC, SU * 512], F8, kind="ExternalInput").ap()
    xgs = [
        nc.dram_tensor(f"xg{i}", [NQ, KC, 2048], F8, kind="ExternalInput").ap()
        for i in range(NG)
    ]
    w1a = nc.dram_tensor(
        "w1a", [KC, NQ * 4 * NCH * Z // 2], F8, kind="ExternalInput"
    ).ap()
    w1b = nc.dram_tensor(
        "w1b", [KC, NQ * 4 * NCH * Z // 2], F8, kind="ExternalInput"
    ).ap()
    # w2c[32j+z, 40q+10j+m] = W2[4q+j, m, z] / (1.1 * softplus(beta))
    w2q = nc.dram_tensor("w2q", [P, NQ * 40], F16, kind="ExternalInput").ap()
    # scal[:, 0:NQ]=softplus(beta), [NQ:2NQ]=sp*b1, [2NQ:3NQ]=b2 (compact, 40 rows)
    scalq = nc.dram_tensor("scalq", [P, 3 * NQ], F32, kind="ExternalInput").ap()
    # maskc[p, m] = 1 iff p//10 == m//10  (p, m < 40)
    maskb = nc.dram_tensor("maskb", [40, 40], BF16, kind="ExternalInput").ap()
    out = nc.dram_tensor("out", [GPC * C, L], BF16, kind="ExternalOutput").ap()

    # block (c, j) -> (bucket, byte offset); canonical block index 4c+j.
    # Blocks 0..SU-1 ride the sync queue (bucket 0, prefetched); the rest
    # split into NG one-chunk granules on gpsimd (buckets 1..NG).
    def cj_off(c, j):
        b = 4 * c + j
        if b < SU:
            return 0, 512 * b
        return 1 + (b - SU) // 4, 512 * ((b - SU) % 4)

    with tile.TileContext(nc) as tc, ExitStack() as ctx:
        consts = ctx.enter_context(tc.tile_pool(name="consts", bufs=1))
        xpool = ctx.enter_context(tc.tile_pool(name="x", bufs=cfg["x_bufs"]))
        spool = ctx.enter_context(tc.tile_pool(name="act", bufs=cfg["s_bufs"]))
        hps = ctx.enter_context(
            tc.tile_pool(name="hps", bufs=cfg["h_bufs"], space="PSUM")
        )
        ops = ctx.enter_context(
            tc.tile_pool(name="ops", bufs=cfg["o_bufs"], space="PSUM")
        )
        tps = ctx.enter_context(tc.tile_pool(name="tps", bufs=2, space="PSUM"))

        xe = _eng(nc, cfg["x_engine"])
        ae = _eng(nc, cfg["aux_engine"])
        oe = _eng(nc, cfg["out_engine"])

        # W1 resident in SBUF on the sync queue: first half (quads 0-3)
        # before any x share so the pipeline can start, second half after
        # the first prefetches (it is only needed ~4 quads in).
        whalf = NQ * 4 * NCH * Z // 2
        w1t = consts.tile([KC, NQ * 4 * NCH * Z], F8, name="w1t")
        ae.dma_start(w1t[:, :whalf], w1a)
        w2t = consts.tile([P, NQ * 40], F16, name="w2t")
        scalt = consts.tile([P, 3 * NQ], F32, name="scalt")
        maskt = consts.tile([40, 40], BF16, name="maskt")

        def load_consts():
            ae.dma_start(w2t[:], w2q)
            ae.dma_start(scalt[:], scalq)
            ae.dma_start(maskt[:], maskb)
            ae.dma_start(w1t[:, whalf:], w1b)
        spht = scalt[:, 0:NQ]
        spb1t = scalt[:, NQ : 2 * NQ]
        b2t = scalt[:, 2 * NQ : 3 * NQ]

        hqs, swishes, expos, esums, invcs = {}, {}, {}, {}, {}

        def w1s(q, j, c):
            k = (q * 4 + j) * NCH + c
            return w1t[:, k * Z : (k + 1) * Z]

        xstiles = {}
        ROT = 3  # tag rotation: a tag's issue-wait references the DMA
        # from ROT quads ago (long complete), so dma_start never blocks
        # the issuing engine and the ring stays full

        def issue_xs(q):
            """Prefetch quad q's sync-queue share (one DMA)."""
            xt = xpool.tile(
                [KC, SU * 512], F8, tag=f"xs{q % ROT}", name=f"xs{q}"
            )
            ae.dma_start(xt[:], xs[q])
            xstiles[q] = xt

        def granules():
            groups = [[] for _ in range(1 + NG)]
            for c in range(NCH):
                for j in range(4):
                    bi, _ = cj_off(c, j)
                    groups[bi].append((c, j))
            return groups

        GROUPS = granules()

        def stage1_dma(q):
            tiles = [xstiles.pop(q)]
            for gi in range(NG):
                xt = xpool.tile(
                    [KC, 2048], F8, tag=f"x{gi}_{q % ROT}", name=f"x{q}_{gi}"
                )
                xe.dma_start(xt[:], xgs[gi][q])
                tiles.append(xt)
            hq = hps.tile([P, L], F32, tag="h", name=f"h{q}")
            hqs[q] = hq
            return tiles, hq

        def mm_block(q, bi, tiles, hq, first, last):
            for c, j in GROUPS[bi]:
                _, off = cj_off(c, j)
                nc.tensor.matmul(
                    hq[32 * j : 32 * j + 32, :],
                    w1s(q, j, c),
                    tiles[bi][:, off : off + 512],
                    start=((bi, c) == first[j]),
                    stop=((bi, c) == last[j]),
                    tile_position=(0, 32 * j),
                    skip_group_check=True,
                )

        def stage_swish(q):
            """GroupSwish as one op: silu(sp*(h+b1)); the 1/(1.1*sp)
            factor is folded into W2 host-side."""
            hq = hqs.pop(q)
            sw = spool.tile([P, L], F16, tag="sw", name=f"sw{q}")
            nc.scalar.activation(
                sw[:],
                hq[:],
                mybir.ActivationFunctionType.Silu,
                bias=spb1t[:, q : q + 1],
                scale=spht[:, q : q + 1],
            )
            swishes[q] = sw

        def stage2(q):
            """Block-diagonal W2 matmul (compact [40, L] output) + exp."""
            sw = swishes.pop(q)
            o = ops.tile([40, L], F32, tag="o", name=f"o{q}")
            nc.tensor.matmul(
                o[:],
                w2t[:, q * 40 : (q + 1) * 40],
                sw[:],
                start=True,
                stop=True,
            )
            expo = spool.tile([40, L], F32, tag="expo", name=f"e{q}")
            esum = spool.tile([40, 1], BF16, tag="esum", name=f"es{q}")
            with nc.allow_low_precision(reason="softmax denom, 2e-2 gate"):
                nc.scalar.activation(
                    expo[:],
                    o[:],
                    mybir.ActivationFunctionType.Exp,
                    bias=b2t[0:40, q : q + 1],
                    scale=1.0,
                    accum_out=esum[:],
                )
            expos[q] = expo
            esums[q] = esum

        def stage3a(q):
            """Per-group exp totals + reciprocal."""
            esum = esums.pop(q)
            tot = tps.tile([40, 1], F32, tag="tot", name=f"tot{q}")
            nc.tensor.matmul(tot[:], maskt[:], esum[:], start=True, stop=True)
            invc = spool.tile([40, 1], F32, tag="invc", name=f"ic{q}")
            nc.vector.reciprocal(invc[:], tot[:])
            invcs[q] = invc

        def stage3b(q):
            """Normalize + store: one plain [40, 512] DMA."""
            invc = invcs.pop(q)
            expo = expos.pop(q)
            res = spool.tile([40, L], BF16, tag="res", name=f"r{q}")
            nc.vector.tensor_scalar_mul(res[:], expo[:], invc[:])
            oe.dma_start(out[40 * q : 40 * q + 40], res[:])

        PF = cfg["prefetch"]
        for q in range(NQ + 3):
            if q == 0:
                for k in range(min(PF + 1, NQ)):
                    issue_xs(k)
                load_consts()
            elif q + PF < NQ:
                issue_xs(q + PF)
            if q < NQ:
                tiles, hq = stage1_dma(q)
                first, last = {}, {}
                for bi in range(1 + NG):
                    for c, j in GROUPS[bi]:
                        key = (bi, c)
                        if j not in first:
                            first[j] = key
                        last[j] = key
                inject = {
                    1: (lambda: stage2(q - 1)) if q >= 1 else None,
                    2: (lambda: stage3a(q - 2)) if q >= 2 else None,
                    3: (lambda: stage3b(q - 3)) if q >= 3 else None,
                }
                for bi in range(1 + NG):
                    mm_block(q, bi, tiles, hq, first, last)
                    cb = inject.pop(bi + 1, None)
                    if cb:
                        cb()
                for cb in inject.values():
                    if cb:
                        cb()
                stage_swish(q)
            elif q == NQ:
                stage2(q - 1)
                stage3a(q - 2)
                stage3b(q - 3)
            elif q == NQ + 1:
                stage3a(q - 2)
                stage3b(q - 3)
            else:
                stage3b(q - 3)

    nc.compile()
    return nc


def _marshal(x, W1, b1, beta, W2, b2, cfg=DEFAULT_CFG):
    """Full inputs -> list of per-core input dicts."""
    fp8 = ml_dtypes.float8_e4m3
    # x: [1, B*X, L] -> [B, 7, 112, L] (g, c, p, l), cast once
    xg8 = np.asarray(x, dtype=np.float32).reshape(B, NCH, KC, L).astype(fp8)
    w1T = np.asarray(W1, dtype=np.float32).transpose(0, 2, 1)  # [B, X, Z]
    w1g = w1T.reshape(B, NCH, KC, Z)  # (g, c, p, z)
    b1f = np.asarray(b1, dtype=np.float32)  # [B, Z]
    b2f = np.asarray(b2, dtype=np.float32)  # [B, C]
    bf = np.asarray(beta, dtype=np.float32)  # [B]
    sph = np.log1p(np.exp(bf)).astype(np.float32)  # softplus(beta), [B]
    # W2 / (1.1 * sp) : [B, C, Z]
    w2s = np.asarray(W2, dtype=np.float32) / (1.1 * sph)[:, None, None]

    pp = np.arange(40)
    maskb = (pp[:, None] // C == pp[None, :] // C).astype(ml_dtypes.bfloat16)

    in_maps = []
    for core in range(NCORE):
        s = slice(core * GPC, (core + 1) * GPC)
        # x -> (q, p, c, j, l) flattened: canonical c-major block order;
        # first SU blocks ride the sync queue, the rest split into NG
        # one-chunk granules (each its own contiguous tensor)
        xfull = (
            xg8[s]
            .reshape(NQ, 4, NCH, KC, L)
            .transpose(0, 3, 2, 1, 4)
            .reshape(NQ, KC, NB * 512)
        )
        xsm = np.ascontiguousarray(xfull[:, :, : SU * 512])
        xgm = [
            np.ascontiguousarray(
                xfull[:, :, (SU + 4 * i) * 512 : (SU + 4 * i + 4) * 512]
            )
            for i in range(NG)
        ]
        # w1q[p, ((q*4+j)*7+c)*Z+z] = W1T[4q+j, 112c+p, z], split in two
        # contiguous halves
        wc = w1g[s].reshape(NQ, 4, NCH, KC, Z)
        w1qm = (
            wc.transpose(3, 0, 1, 2, 4).astype(fp8).reshape(KC, NQ * 4 * NCH * Z)
        )
        whalf = NQ * 4 * NCH * Z // 2
        w1am = np.ascontiguousarray(w1qm[:, :whalf])
        w1bm = np.ascontiguousarray(w1qm[:, whalf:])
        # w2c[32j+z, 40q+10j+m] = w2s[4q+j, m, z]
        w2c = w2s[s].reshape(NQ, 4, C, Z)  # (q, j, m, z)
        w2qm = np.zeros((4, Z, NQ, 4, C), np.float16)
        for j in range(4):
            w2qm[j, :, :, j, :] = w2c[:, j].transpose(2, 0, 1)  # (z, q, m)
        w2qm = w2qm.reshape(P, NQ * 40)
        # per-partition scalars
        sphq = np.ascontiguousarray(
            np.broadcast_to(
                sph[s].reshape(NQ, 4).T[:, None, :], (4, Z, NQ)
            )
        ).reshape(P, NQ)
        b1q = np.ascontiguousarray(
            b1f[s].reshape(NQ, 4, Z).transpose(1, 2, 0)
        ).reshape(P, NQ)
        spb1q = sphq * b1q
        b2q = np.zeros((P, NQ), np.float32)
        b2q[0:40] = (
            b2f[s].reshape(NQ, 4, C).transpose(1, 2, 0).reshape(40, NQ)
        )
        scalqm = np.concatenate([sphq, spb1q, b2q], axis=1)
        im = {
            "xs": xsm,
            "w1a": w1am,
            "w1b": w1bm,
            "w2q": w2qm,
            "scalq": scalqm,
            "maskb": maskb,
        }
        for i in range(NG):
            im[f"xg{i}"] = xgm[i]
        in_maps.append(im)
    return in_maps


def _run(in_maps, cfg=DEFAULT_CFG, trace=False, tmpdir=None):
    key = str(sorted(cfg.items()))
    if key not in _CACHE:
        _CACHE[key] = _build(cfg)
    return run_bass_kernel_spmd(
        _CACHE[key],
        in_maps,
        core_ids=list(range(NCORE)),
        trace=trace,
        tmpdir=tmpdir,
    )


_LAST = {}


def kernel(x, W1, b1, beta, W2, b2):
    cfg = dict(DEFAULT_CFG)
    ov = os.environ.get("KERNEL_CFG")
    if ov:
        for kv in ov.split(","):
            k, v = kv.split("=")
            cfg[k] = type(DEFAULT_CFG[k])(eval(v)) if not isinstance(
                DEFAULT_CFG[k], str
            ) else v
    in_maps = _marshal(x, W1, b1, beta, W2, b2, cfg)
    trace = bool(os.environ.get("KERNEL_TRACE"))
    r = _run(in_maps, cfg, trace=trace, tmpdir=os.environ.get("KERNEL_TRACE_DIR"))
    _LAST["results"] = r
    outs = [
        r.results[c]["out"].astype(np.float32).reshape(GPC, C * L)
        for c in range(NCORE)
    ]
    return np.concatenate(outs, axis=0)
